# revision 25
# baseline (speedup 1.0000x reference)
"""Multi-head attention (B=4, S=2048, D=1024, H=16, dk=dv=64) on 8 TRN2 cores.

Sharding: core = (batch b, head-group g): data-parallel over batch (4) x
tensor-parallel over heads (2 groups of 8). Each core computes its batch's
Q/K/V projections for its 8 heads, attention, and a partial output
projection over its heads' rows of Wo. The host sums the two partial
outputs per batch.

Per-core kernel (matmul inputs float32r = TF32-like, fp32 accumulate):
  V phase:  xT streamed in s-tiles of 512; V for all 8 heads projected into
            a [s, head, 64+1] layout whose per-head ones column later yields
            softmax denominators for free.
  pair loop (4 head pairs): Q/K projected pair-packed (two heads' 64 dk dims
            on partitions 0:64 / 64:128, xT re-streamed), then attention:
            per 512-q-block, both heads' scores land in one [128,1024] PSUM
            tile via matmuls contracting disjoint row-groups (concurrent on
            the PE), one ACT exp (scale=1/8 folded in) covers both, AV
            accumulates per head over 16 s-chunks; row 64 of the AV PSUM is
            the softmax denominator. Normalize = DVE reciprocal + GPSIMD
            partition broadcast + DVE multiply.
  out phase: output projection from the normalized [hv, s] head layout.
"""

import numpy as np

import concourse.bacc as bacc
import concourse.tile as tile
import concourse.mybir as mybir
from concourse.bass_utils import run_bass_kernel_spmd

F32 = mybir.dt.float32
F32R = mybir.dt.float32r
EXP = mybir.ActivationFunctionType.Exp

P = 128
S = 2048
D = 1024
DK = 64
HPC = 8            # heads per core
NSC = S // P       # 16 s-chunks of 128
NST = 4            # s-tiles of 512
STW = S // NST     # 512
NDC = D // P       # 8 d_model chunks
NPAIR = HPC // 2   # 4 head pairs
NQB = S // STW     # 4 q-blocks of 512 in attention
SCALE = 1.0 / np.sqrt(DK)


def build_kernel():
    nc = bacc.Bacc("TRN2", target_bir_lowering=False, debug=False)

    xt_d = nc.dram_tensor("xt", [D, S], F32R, kind="ExternalInput")
    wq_d = nc.dram_tensor("wq", [NPAIR, D, P], F32R, kind="ExternalInput")
    wk_d = nc.dram_tensor("wk", [NPAIR, D, P], F32R, kind="ExternalInput")
    wv_d = nc.dram_tensor("wv", [D, HPC * DK], F32R, kind="ExternalInput")
    wo_d = nc.dram_tensor("wo", [HPC * DK, D], F32R, kind="ExternalInput")
    ones_d = nc.dram_tensor("ones", [P, P], F32R, kind="ExternalInput")
    out_d = nc.dram_tensor("out", [S, D], F32, kind="ExternalOutput")

    xt_ap = xt_d.ap().rearrange("(dc p) s -> p dc s", p=P)

    with tile.TileContext(nc) as tc:
        with tc.tile_pool(name="persist", bufs=1) as persist, \
             tc.tile_pool(name="xtp", bufs=2) as xtp, \
             tc.tile_pool(name="stage", bufs=4) as stage:
            # V with ones column per head: [128 s, sc, head, 64+1]
            v520 = persist.tile([P, NSC, HPC, DK + 1], F32R, tag="v520")
            # normalized heads, [hv, s]: chunk ci = heads (2ci, 2ci+1)
            hn = persist.tile([P, NPAIR, S], F32R, tag="hn")
            wo_sb = persist.tile([P, NPAIR, D], F32R, tag="wo")
            ones_sb = persist.tile([P, HPC], F32R, tag="ones_sb")

            # -------- per-pair: QK projection + attention --------
            # Pair 0's first pass also projects V (shared xT stream), with
            # the pair-0 Q/K matmuls issued first so the first scores/exp
            # start as early as possible.
            with tc.tile_pool(name="wvp", bufs=1) as wvp, \
                 tc.tile_pool(name="qkpool", bufs=2) as qkpool, \
                 tc.tile_pool(name="wqkp", bufs=2) as wqkp, \
                 tc.tile_pool(name="expp", bufs=2) as expp, \
                 tc.tile_pool(name="smallp", bufs=2) as smallp, \
                 tc.tile_pool(name="bcsb", bufs=2) as bcsb, \
                 tc.tile_pool(name="qkps", bufs=2, space="PSUM") as qkps, \
                 tc.tile_pool(name="scps", bufs=2, space="PSUM") as scps, \
                 tc.tile_pool(name="avps", bufs=1, space="PSUM") as avps:
                vps = qkps  # shared double-buffered [128, 512] psum tag
                wv_sb = wvp.tile([P, NDC, HPC * DK], F32R, tag="wv")
                for pr in range(NPAIR):
                    wqp = wqkp.tile([P, NDC, P], F32R, tag="wqp")
                    wkp = wqkp.tile([P, NDC, P], F32R, tag="wkp")
                    nc.sync.dma_start(
                        wqp[:],
                        wq_d.ap()[pr].rearrange("(dc p) c -> p dc c", p=P))
                    nc.sync.dma_start(
                        wkp[:],
                        wk_d.ap()[pr].rearrange("(dc p) c -> p dc c", p=P))
                    if pr == 0:
                        # needed from the first pass, but after pair-0 Q/K
                        nc.sync.dma_start(ones_sb[:], ones_d.ap()[:, 0:HPC])
                        nc.sync.dma_start(
                            wv_sb[:],
                            wv_d.ap().rearrange("(dc p) c -> p dc c", p=P))
                    elif pr == 1:
                        # only needed by the output projection at the end
                        nc.sync.dma_start(
                            wo_sb[:],
                            wo_d.ap().rearrange("(ci p) d -> p ci d", p=P))
                    qtp = qkpool.tile([P, S], F32R, tag="qt")
                    ktp = qkpool.tile([P, S], F32R, tag="kt")

                    def out_group(sc, dmh):
                        # one output-projection psum group ([128, 512] out
                        # rows sc, cols dmh-half); reuses the qkp PSUM banks
                        ps = qkps.tile([P, D // 2], F32, tag="qkp",
                                       name="ops")
                        for ci in range(NPAIR):
                            nc.tensor.matmul(
                                ps[:], hn[:, ci, sc * P:(sc + 1) * P],
                                wo_sb[:, ci, dmh * 512:(dmh + 1) * 512],
                                start=(ci == 0), stop=(ci == NPAIR - 1))
                        osb = stage.tile([P, D // 2], F32,
                                         tag="ostage", name="osb")
                        nc.vector.tensor_copy(osb[:], ps[:])
                        nc.sync.dma_start(
                            out_d.ap()[sc * P:(sc + 1) * P,
                                       dmh * 512:(dmh + 1) * 512],
                            osb[:])

                    def attn_chunk(pr, qb, sc_lo, sc_hi, av, trail=None,
                                   qtp=qtp, ktp=ktp):
                        q0 = qb * STW
                        for sc in range(sc_lo, sc_hi):
                            scp = scps.tile([P, 2 * STW], F32, tag="scp",
                                            name="scp")
                            for j in range(2):
                                nc.tensor.matmul(
                                    scp[:, j * STW:(j + 1) * STW],
                                    ktp[j * DK:(j + 1) * DK,
                                        sc * P:(sc + 1) * P],
                                    qtp[j * DK:(j + 1) * DK, q0:q0 + STW],
                                    start=True, stop=True)
                            ex = expp.tile([P, 2 * STW], F32R, tag="exp",
                                           name="ex")
                            nc.scalar.activation(
                                ex[:], scp[:], EXP, scale=float(SCALE))
                            for j in range(2):
                                nc.tensor.matmul(
                                    av[j][:], v520[:, sc, 2 * pr + j, :],
                                    ex[:, j * STW:(j + 1) * STW],
                                    start=(sc == 0), stop=(sc == NSC - 1),
                                    skip_group_check=True)
                            if trail and sc % 2 == 1:
                                out_group(*trail.pop(0))

                    def attn_norm(pr, qb, av):
                        q0 = qb * STW
                        for j in range(2):
                            # copy PSUM->SBUF first so the AV bank frees
                            # immediately; normalize off the critical path
                            avs = bcsb.tile([DK + 1, STW], F32, tag="avs",
                                            name="avs")
                            nc.vector.tensor_copy(avs[:], av[j][:])
                            rec = smallp.tile([1, STW], F32R, tag="rec",
                                              name="rec")
                            with nc.allow_low_precision(
                                    reason="softmax recip feeds fp32r mm"):
                                nc.vector.reciprocal(
                                    rec[:], avs[DK:DK + 1, :])
                            bcs = bcsb.tile([DK, STW], F32R, tag="bcs",
                                            name="bcs")
                            nc.gpsimd.partition_broadcast(
                                bcs[:], rec[:], channels=DK)
                            nc.vector.tensor_mul(
                                hn[j * DK:(j + 1) * DK, pr, q0:q0 + STW],
                                avs[0:DK, :], bcs[:])

                    def new_av():
                        return [avps.tile([DK + 1, STW], F32, tag=f"av{j}",
                                          name=f"av{j}")
                                for j in range(2)]

                    av0 = new_av() if pr == 0 else None
                    for st in range(NST):
                        xts = xtp.tile([P, NDC, STW], F32R, tag="xts")
                        for dh in range(0, NDC, 2):
                            nc.sync.dma_start(
                                xts[:, dh:dh + 2, :],
                                xt_ap[:, dh:dh + 2,
                                      st * STW:(st + 1) * STW])
                        for w_sb, dst in ((wqp, qtp), (wkp, ktp)):
                            ps = qkps.tile([P, STW], F32, tag="qkp")
                            for dc in range(NDC):
                                nc.tensor.matmul(
                                    ps[:], w_sb[:, dc, :], xts[:, dc, :],
                                    start=(dc == 0), stop=(dc == NDC - 1))
                            nc.vector.tensor_copy(
                                dst[:, st * STW:(st + 1) * STW], ps[:])
                        if pr == 0:
                            # V projection rides pair 0's xT stream
                            for scl in range(STW // P):
                                sc = st * (STW // P) + scl
                                ps = vps.tile([P, HPC * DK], F32, tag="qkp")
                                for dc in range(NDC):
                                    nc.tensor.matmul(
                                        ps[:],
                                        xts[:, dc, scl * P:(scl + 1) * P],
                                        wv_sb[:, dc, :],
                                        start=(dc == 0), stop=(dc == NDC - 1))
                                nc.vector.tensor_copy(
                                    v520[:, sc, :, 0:DK],
                                    ps.rearrange("p (h v) -> p h v", v=DK))
                                nc.vector.tensor_copy(
                                    v520[:, sc, :, DK:DK + 1],
                                    ones_sb[:, :, None])
                            # pair-0 q-block 0 starts as soon as this st's
                            # K/V chunks exist (queries 0:512 are st 0)
                            attn_chunk(0, 0, st * 4, (st + 1) * 4, av0)

                    # attention for this pair, per 512-wide q-block. For the
                    # last pair, q-block qb-1 is complete once norm(qb-1)
                    # ran, so its output-projection groups interleave into
                    # attention of qb (one group per two s-chunks).
                    if pr == 0:
                        attn_norm(0, 0, av0)
                    for qb in range(1 if pr == 0 else 0, NQB):
                        av = new_av()
                        trail = None
                        if pr == NPAIR - 1 and qb > 0:
                            trail = [(sc, dmh)
                                     for sc in range((qb - 1) * 4, qb * 4)
                                     for dmh in range(2)]
                        attn_chunk(pr, qb, 0, NSC, av, trail=trail)
                        attn_norm(pr, qb, av)
                    if pr == NPAIR - 1:
                        for sc in range((NQB - 1) * 4, NQB * 4):
                            for dmh in range(2):
                                out_group(sc, dmh)

    nc.compile()
    return nc


_NC_CACHE = None


def _get_nc():
    global _NC_CACHE
    if _NC_CACHE is None:
        _NC_CACHE = build_kernel()
    return _NC_CACHE


def kernel(x, Wq, Wk, Wv, Wo):
    x = np.asarray(x, dtype=np.float32)
    Wq = np.asarray(Wq, dtype=np.float32)
    Wk = np.asarray(Wk, dtype=np.float32)
    Wv = np.asarray(Wv, dtype=np.float32)
    Wo = np.asarray(Wo, dtype=np.float32)
    B = x.shape[0]
    ones = np.ones((P, P), dtype=np.float32)

    in_maps = []
    for core in range(8):
        b, g = divmod(core, 2)
        hs = g * HPC
        xt = np.ascontiguousarray(x[b].T)
        wq = np.stack([
            np.concatenate([Wq[hs + 2 * p], Wq[hs + 2 * p + 1]], axis=1)
            for p in range(NPAIR)])
        wk = np.stack([
            np.concatenate([Wk[hs + 2 * p], Wk[hs + 2 * p + 1]], axis=1)
            for p in range(NPAIR)])
        wv = np.concatenate([Wv[hs + h] for h in range(HPC)], axis=1)
        wo = np.ascontiguousarray(Wo[hs * DK:(hs + HPC) * DK, :])
        in_maps.append({"xt": xt, "wq": wq, "wk": wk, "wv": wv, "wo": wo,
                        "ones": ones})

    nc = _get_nc()
    res = run_bass_kernel_spmd(nc, in_maps, core_ids=list(range(8))).results

    out = np.empty((B, S, D), dtype=np.float32)
    for b in range(B):
        out[b] = res[2 * b]["out"] + res[2 * b + 1]["out"]
    return out


# revision 26
# speedup vs baseline: 1.0001x; 1.0001x over previous
"""Multi-head attention (B=4, S=2048, D=1024, H=16, dk=dv=64) on 8 TRN2 cores.

Sharding: core = (batch b, head-group g): data-parallel over batch (4) x
tensor-parallel over heads (2 groups of 8). Each core computes its batch's
Q/K/V projections for its 8 heads, attention, and a partial output
projection over its heads' rows of Wo. The host sums the two partial
outputs per batch.

Per-core kernel (matmul inputs float32r = TF32-like, fp32 accumulate):
  V phase:  xT streamed in s-tiles of 512; V for all 8 heads projected into
            a [s, head, 64+1] layout whose per-head ones column later yields
            softmax denominators for free.
  pair loop (4 head pairs): Q/K projected pair-packed (two heads' 64 dk dims
            on partitions 0:64 / 64:128, xT re-streamed), then attention:
            per 512-q-block, both heads' scores land in one [128,1024] PSUM
            tile via matmuls contracting disjoint row-groups (concurrent on
            the PE), one ACT exp (scale=1/8 folded in) covers both, AV
            accumulates per head over 16 s-chunks; row 64 of the AV PSUM is
            the softmax denominator. Normalize = DVE reciprocal + GPSIMD
            partition broadcast + DVE multiply.
  out phase: output projection from the normalized [hv, s] head layout.
"""

import numpy as np

import concourse.bacc as bacc
import concourse.tile as tile
import concourse.mybir as mybir
from concourse.bass_utils import run_bass_kernel_spmd

F32 = mybir.dt.float32
F32R = mybir.dt.float32r
EXP = mybir.ActivationFunctionType.Exp

P = 128
S = 2048
D = 1024
DK = 64
HPC = 8            # heads per core
NSC = S // P       # 16 s-chunks of 128
NST = 4            # s-tiles of 512
STW = S // NST     # 512
NDC = D // P       # 8 d_model chunks
NPAIR = HPC // 2   # 4 head pairs
NQB = S // STW     # 4 q-blocks of 512 in attention
SCALE = 1.0 / np.sqrt(DK)


def build_kernel():
    nc = bacc.Bacc("TRN2", target_bir_lowering=False, debug=False)

    xt_d = nc.dram_tensor("xt", [D, S], F32R, kind="ExternalInput")
    wq_d = nc.dram_tensor("wq", [NPAIR, D, P], F32R, kind="ExternalInput")
    wk_d = nc.dram_tensor("wk", [NPAIR, D, P], F32R, kind="ExternalInput")
    wv_d = nc.dram_tensor("wv", [D, HPC * DK], F32R, kind="ExternalInput")
    wo_d = nc.dram_tensor("wo", [HPC * DK, D], F32R, kind="ExternalInput")
    ones_d = nc.dram_tensor("ones", [P, P], F32R, kind="ExternalInput")
    out_d = nc.dram_tensor("out", [S, D], F32, kind="ExternalOutput")

    xt_ap = xt_d.ap().rearrange("(dc p) s -> p dc s", p=P)

    with tile.TileContext(nc) as tc:
        with tc.tile_pool(name="persist", bufs=1) as persist, \
             tc.tile_pool(name="xtp", bufs=2) as xtp, \
             tc.tile_pool(name="stage", bufs=4) as stage:
            # V with ones column per head: [128 s, sc, head, 64+1]
            v520 = persist.tile([P, NSC, HPC, DK + 1], F32R, tag="v520")
            # normalized heads, [hv, s]: chunk ci = heads (2ci, 2ci+1)
            hn = persist.tile([P, NPAIR, S], F32R, tag="hn")
            wo_sb = persist.tile([P, NPAIR, D], F32R, tag="wo")
            ones_sb = persist.tile([P, HPC], F32R, tag="ones_sb")

            # -------- per-pair: QK projection + attention --------
            # Pair 0's first pass also projects V (shared xT stream), with
            # the pair-0 Q/K matmuls issued first so the first scores/exp
            # start as early as possible.
            with tc.tile_pool(name="wvp", bufs=1) as wvp, \
                 tc.tile_pool(name="qkpool", bufs=2) as qkpool, \
                 tc.tile_pool(name="wqkp", bufs=2) as wqkp, \
                 tc.tile_pool(name="expp", bufs=2) as expp, \
                 tc.tile_pool(name="smallp", bufs=2) as smallp, \
                 tc.tile_pool(name="bcsb", bufs=2) as bcsb, \
                 tc.tile_pool(name="qkps", bufs=2, space="PSUM") as qkps, \
                 tc.tile_pool(name="scps", bufs=2, space="PSUM") as scps, \
                 tc.tile_pool(name="avps", bufs=1, space="PSUM") as avps:
                vps = qkps  # shared double-buffered [128, 512] psum tag
                wv_sb = wvp.tile([P, NDC, HPC * DK], F32R, tag="wv")
                for pr in range(NPAIR):
                    wqp = wqkp.tile([P, NDC, P], F32R, tag="wqp")
                    wkp = wqkp.tile([P, NDC, P], F32R, tag="wkp")
                    nc.sync.dma_start(
                        wqp[:],
                        wq_d.ap()[pr].rearrange("(dc p) c -> p dc c", p=P))
                    nc.sync.dma_start(
                        wkp[:],
                        wk_d.ap()[pr].rearrange("(dc p) c -> p dc c", p=P))
                    if pr == 0:
                        # needed from the first pass, but after pair-0 Q/K
                        nc.sync.dma_start(ones_sb[:], ones_d.ap()[:, 0:HPC])
                        nc.sync.dma_start(
                            wv_sb[:],
                            wv_d.ap().rearrange("(dc p) c -> p dc c", p=P))
                    elif pr == 1:
                        # only needed by the output projection at the end
                        nc.sync.dma_start(
                            wo_sb[:],
                            wo_d.ap().rearrange("(ci p) d -> p ci d", p=P))
                    qtp = qkpool.tile([P, S], F32R, tag="qt")
                    ktp = qkpool.tile([P, S], F32R, tag="kt")

                    def out_group(sc, dmh):
                        # one output-projection psum group ([128, 512] out
                        # rows sc, cols dmh-half); reuses the qkp PSUM banks
                        ps = qkps.tile([P, D // 2], F32, tag="qkp",
                                       name="ops")
                        for ci in range(NPAIR):
                            nc.tensor.matmul(
                                ps[:], hn[:, ci, sc * P:(sc + 1) * P],
                                wo_sb[:, ci, dmh * 512:(dmh + 1) * 512],
                                start=(ci == 0), stop=(ci == NPAIR - 1))
                        osb = stage.tile([P, D // 2], F32,
                                         tag="ostage", name="osb")
                        nc.vector.tensor_copy(osb[:], ps[:])
                        nc.sync.dma_start(
                            out_d.ap()[sc * P:(sc + 1) * P,
                                       dmh * 512:(dmh + 1) * 512],
                            osb[:])

                    def attn_chunk(pr, qb, sc_lo, sc_hi, av, trail=None,
                                   qtp=qtp, ktp=ktp):
                        q0 = qb * STW
                        for sc in range(sc_lo, sc_hi):
                            scp = scps.tile([P, 2 * STW], F32, tag="scp",
                                            name="scp")
                            for j in range(2):
                                nc.tensor.matmul(
                                    scp[:, j * STW:(j + 1) * STW],
                                    ktp[j * DK:(j + 1) * DK,
                                        sc * P:(sc + 1) * P],
                                    qtp[j * DK:(j + 1) * DK, q0:q0 + STW],
                                    start=True, stop=True)
                            ex = expp.tile([P, 2 * STW], F32R, tag="exp",
                                           name="ex")
                            nc.scalar.activation(
                                ex[:], scp[:], EXP, scale=float(SCALE))
                            for j in range(2):
                                nc.tensor.matmul(
                                    av[j][:], v520[:, sc, 2 * pr + j, :],
                                    ex[:, j * STW:(j + 1) * STW],
                                    start=(sc == 0), stop=(sc == NSC - 1),
                                    skip_group_check=True)
                            if trail and sc % 2 == 1:
                                out_group(*trail.pop(0))

                    def attn_norm(pr, qb, av):
                        q0 = qb * STW
                        for j in range(2):
                            # copy PSUM->SBUF first so the AV bank frees
                            # immediately; normalize off the critical path
                            avs = bcsb.tile([DK + 1, STW], F32, tag="avs",
                                            name="avs")
                            nc.vector.tensor_copy(avs[:], av[j][:])
                            rec = smallp.tile([1, STW], F32R, tag="rec",
                                              name="rec")
                            with nc.allow_low_precision(
                                    reason="softmax recip feeds fp32r mm"):
                                nc.vector.reciprocal(
                                    rec[:], avs[DK:DK + 1, :])
                            bcs = bcsb.tile([DK, STW], F32R, tag="bcs",
                                            name="bcs")
                            nc.gpsimd.partition_broadcast(
                                bcs[:], rec[:], channels=DK)
                            nc.vector.tensor_mul(
                                hn[j * DK:(j + 1) * DK, pr, q0:q0 + STW],
                                avs[0:DK, :], bcs[:])

                    def new_av():
                        return [avps.tile([DK + 1, STW], F32, tag=f"av{j}",
                                          name=f"av{j}")
                                for j in range(2)]

                    av0 = new_av() if pr == 0 else None
                    for st in range(NST):
                        xts = xtp.tile([P, NDC, STW], F32R, tag="xts")
                        for dh in range(0, NDC, 2):
                            nc.sync.dma_start(
                                xts[:, dh:dh + 2, :],
                                xt_ap[:, dh:dh + 2,
                                      st * STW:(st + 1) * STW])
                        for w_sb, dst in ((wqp, qtp), (wkp, ktp)):
                            ps = qkps.tile([P, STW], F32, tag="qkp")
                            for dc in range(NDC):
                                nc.tensor.matmul(
                                    ps[:], w_sb[:, dc, :], xts[:, dc, :],
                                    start=(dc == 0), stop=(dc == NDC - 1))
                            nc.vector.tensor_copy(
                                dst[:, st * STW:(st + 1) * STW], ps[:])
                        if pr < 2:
                            # V projection rides the first two pairs' xT
                            # streams, half the heads each (N=256 keeps the
                            # fp32r full-rate >=256 threshold); pair 0 only
                            # needs heads 0-1's V for its own attention.
                            h0 = pr * (HPC // 2)
                            c0 = h0 * DK
                            for scl in range(STW // P):
                                sc = st * (STW // P) + scl
                                ps = vps.tile([P, HPC * DK], F32, tag="qkp")
                                for dc in range(NDC):
                                    nc.tensor.matmul(
                                        ps[:, 0:HPC * DK // 2],
                                        xts[:, dc, scl * P:(scl + 1) * P],
                                        wv_sb[:, dc, c0:c0 + HPC * DK // 2],
                                        start=(dc == 0), stop=(dc == NDC - 1))
                                nc.vector.tensor_copy(
                                    v520[:, sc, h0:h0 + HPC // 2, 0:DK],
                                    ps[:, 0:HPC * DK // 2].rearrange(
                                        "p (h v) -> p h v", v=DK))
                                nc.vector.tensor_copy(
                                    v520[:, sc, h0:h0 + HPC // 2,
                                         DK:DK + 1],
                                    ones_sb[:, h0:h0 + HPC // 2, None])
                        if pr == 0:
                            # pair-0 q-block 0 starts as soon as this st's
                            # K/V chunks exist (queries 0:512 are st 0)
                            attn_chunk(0, 0, st * 4, (st + 1) * 4, av0)

                    # attention for this pair, per 512-wide q-block. For the
                    # last pair, q-block qb-1 is complete once norm(qb-1)
                    # ran, so its output-projection groups interleave into
                    # attention of qb (one group per two s-chunks).
                    if pr == 0:
                        attn_norm(0, 0, av0)
                    for qb in range(1 if pr == 0 else 0, NQB):
                        av = new_av()
                        trail = None
                        if pr == NPAIR - 1 and qb > 0:
                            trail = [(sc, dmh)
                                     for sc in range((qb - 1) * 4, qb * 4)
                                     for dmh in range(2)]
                        attn_chunk(pr, qb, 0, NSC, av, trail=trail)
                        attn_norm(pr, qb, av)
                    if pr == NPAIR - 1:
                        for sc in range((NQB - 1) * 4, NQB * 4):
                            for dmh in range(2):
                                out_group(sc, dmh)

    nc.compile()
    return nc


_NC_CACHE = None


def _get_nc():
    global _NC_CACHE
    if _NC_CACHE is None:
        _NC_CACHE = build_kernel()
    return _NC_CACHE


def kernel(x, Wq, Wk, Wv, Wo):
    x = np.asarray(x, dtype=np.float32)
    Wq = np.asarray(Wq, dtype=np.float32)
    Wk = np.asarray(Wk, dtype=np.float32)
    Wv = np.asarray(Wv, dtype=np.float32)
    Wo = np.asarray(Wo, dtype=np.float32)
    B = x.shape[0]
    ones = np.ones((P, P), dtype=np.float32)

    in_maps = []
    for core in range(8):
        b, g = divmod(core, 2)
        hs = g * HPC
        xt = np.ascontiguousarray(x[b].T)
        wq = np.stack([
            np.concatenate([Wq[hs + 2 * p], Wq[hs + 2 * p + 1]], axis=1)
            for p in range(NPAIR)])
        wk = np.stack([
            np.concatenate([Wk[hs + 2 * p], Wk[hs + 2 * p + 1]], axis=1)
            for p in range(NPAIR)])
        wv = np.concatenate([Wv[hs + h] for h in range(HPC)], axis=1)
        wo = np.ascontiguousarray(Wo[hs * DK:(hs + HPC) * DK, :])
        in_maps.append({"xt": xt, "wq": wq, "wk": wk, "wv": wv, "wo": wo,
                        "ones": ones})

    nc = _get_nc()
    res = run_bass_kernel_spmd(nc, in_maps, core_ids=list(range(8))).results

    out = np.empty((B, S, D), dtype=np.float32)
    for b in range(B):
        out[b] = res[2 * b]["out"] + res[2 * b + 1]["out"]
    return out
